# revision 1
# baseline (speedup 1.0000x reference)
"""Trainium2 Bass kernel for nn_META_RNNDetector: 2-layer LSTM (H=256, I=4)
over B=64, T=2048 with a sliding-window input built from y, followed by a
[256 -> 2] linear head.

Strategy (hardcoded for the shapes above):
  - Data-parallel over batch: 8 cores x 8 batch rows each, LSTM weights
    replicated.
  - Per core, the serial time loop computes gates transposed
    (gates.T: [1024 rows, 8 batch]) so the elementwise LSTM-cell math runs on
    full-width [128, *] tiles. Matmuls keep the (reordered, pre-transposed)
    weights as the PE stationary operand, streamed in bf16 (fp32 PSUM accum).
  - Gate rows are reordered to [i, f, o, g] so one sigmoid covers cols 0:48 of
    the [128, 64] gate view and one tanh covers 48:64.
  - The input-to-hidden contribution of layer 0 (+ its biases) is precomputed
    in parallel over time into an SBUF-resident tensor (bf16), in T/4 chunks
    double-buffered against the serial loop.
  - The final FC is fused into the serial loop as two tiny K=128 matmuls per
    step accumulating [8, 2] logits into PSUM.
"""

import numpy as np
import ml_dtypes

import concourse.bacc as bacc
import concourse.mybir as mybir
import concourse.tile as tile
from concourse.bass import ds, ts
from concourse.bass_utils import run_bass_kernel_spmd

BF16 = mybir.dt.bfloat16
F32 = mybir.dt.float32
AF = mybir.ActivationFunctionType
nbf16 = ml_dtypes.bfloat16

H = 256
NI = 4
T = 2048
B = 64
N_CORES = 8
BL = B // N_CORES          # 8 batch rows per core
U = 16                     # steps per hardware-loop iteration
NQ = 8                     # number of time chunks for G0x staging
QT = T // NQ               # 512 steps per quarter

_compiled = {}


def _gate_perm():
    # torch gate order [i, f, g, o] -> our row order [i, f, o, g]
    idx = np.arange(4 * H).reshape(4, H)
    return np.concatenate([idx[0], idx[1], idx[3], idx[2]])


def build(reps: int = 1, T=T, U=U, NQ=NQ, n_cores=N_CORES, sim=False, probe=''):
    """Build + compile the per-core Bass program. reps>1 wraps the whole
    computation in a repeat loop (identical recompute) for wall-clock timing."""
    QT = T // NQ
    nc = bacc.Bacc("TRN2", target_bir_lowering=False, debug=sim,
                   num_devices=n_cores)

    # ---- I/O ----------------------------------------------------------------
    # host-prepared weight images (bf16) and bias tile (f32); per-core y slice.
    w0_in = nc.dram_tensor("w0img", [128, 16 * 128], BF16, kind="ExternalInput")
    w1_in = nc.dram_tensor("w1img", [128, 32 * 128], BF16, kind="ExternalInput")
    wi0_in = nc.dram_tensor("wi0img", [5, 8 * 128], BF16, kind="ExternalInput")
    wfc_in = nc.dram_tensor("wfcimg", [128, 4], BF16, kind="ExternalInput")
    b1_in = nc.dram_tensor("b1img", [128, 64], BF16, kind="ExternalInput")
    id_in = nc.dram_tensor("identimg", [128, 128], BF16, kind="ExternalInput")
    y_in = nc.dram_tensor("yloc", [BL, T], F32, kind="ExternalInput")
    out_d = nc.dram_tensor("out", [BL, 2 * T + 2 * U], F32, kind="ExternalOutput")

    with tile.TileContext(nc) as tc:
        with (
            tc.tile_pool(name="singles", bufs=1) as singles,
            tc.tile_pool(name="gxpool", bufs=1) as gxpool,
            tc.tile_pool(name="pcps", bufs=1 if "bufs3" in probe else 2, space="PSUM") as pcps,
            tc.tile_pool(name="gps", bufs=3 if "bufs3" in probe else 2, space="PSUM") as gps_pool,
            tc.tile_pool(name="fcps", bufs=1, space="PSUM") as fcps_pool,
            tc.tile_pool(name="sg", bufs=4 if "bufs3" in probe else 2) as sg_pool,
            tc.tile_pool(name="osb", bufs=2) as osb_pool,
        ):
            # ---- persistent SBUF tiles -------------------------------------
            W0 = singles.tile([128, 16 * 128], BF16)   # Whh0' tiles (k*8+m)
            W1 = singles.tile([128, 32 * 128], BF16)   # [Wih1'|Whh1'] tiles
            Wi0 = singles.tile([5, 8 * 128], BF16)     # K=5 (4 taps + bias)
            Wfc = singles.tile([128, 4], BF16)
            b1t = singles.tile([128, 64], BF16)
            ident = singles.tile([128, 128], BF16)
            nc.sync.dma_start(W0[:], w0_in.ap())
            nc.sync.dma_start(W1[:], w1_in.ap())
            nc.sync.dma_start(Wi0[:], wi0_in.ap())
            nc.sync.dma_start(Wfc[:], wfc_in.ap())
            nc.sync.dma_start(b1t[:], b1_in.ap())
            nc.sync.dma_start(ident[:], id_in.ap())

            h0 = singles.tile([128, 16], BF16)   # h0.T as 2 chunks of [128, 8]
            h1a = singles.tile([128, 16], BF16)  # h1(s) lives in [h1a, h1b][s % 2]
            h1b = singles.tile([128, 16], BF16)
            h1bufs = [h1a, h1b]
            c0 = singles.tile([128, 16], F32)
            c1 = singles.tile([128, 16], F32)
            tc0 = singles.tile([128, 16], F32)
            tc1 = singles.tile([128, 16], F32)

            # sliding-window input, transposed: XT[s, b*T + t] = y[b, t-3+s]
            # row 4 == 1.0 (bias lane). bf16.
            XT = singles.tile([5, BL * T], BF16)
            ybf = singles.tile([BL, T], BF16)

            # G0x staging: two quarter-sized buffers [128, QT, 64] bf16
            GX = []
            for j in range(2):
                gxt = gxpool.tile([128, QT, 64], BF16, tag=f"gx{j}", name=f"gx{j}")
                GX.append(gxt)

            def init_state():
                nc.vector.memset(h0[:], 0.0)
                nc.vector.memset(h1a[:], 0.0)
                nc.vector.memset(h1b[:], 0.0)
                nc.vector.memset(c0[:], 0.0)
                nc.vector.memset(c1[:], 0.0)
                # build XT: set all to 1.0 (bias lane keeps it), then the
                # first 4 rows to the -100 pad value before the shifted copies.
                nc.vector.memset(XT[0:5, :], 1.0)
                nc.vector.memset(XT[0:4, :], -100.0)
                ysb = osb_pool.tile([BL, T], F32, tag="ysb", bufs=1)
                nc.sync.dma_start(ysb[:], y_in.ap())
                nc.vector.tensor_copy(ybf[:], ysb[:])
                for s in range(4):
                    n = T - (3 - s)
                    nc.sync.dma_start(
                        XT[s:s + 1, :].rearrange("p (b t) -> p b t", b=BL)[:, :, (3 - s):],
                        ybf[:, 0:n],
                    )

            def precompute_quarter(q, gx):
                """G0x[t, :, :] for t in quarter q -> gx tile [128, QT, 64].
                gx[p, t, 8*m + b] = (Wih0' @ x_t.T + b0')[128m+p, b]"""
                t0 = q * QT
                BLK = min(512, QT)
                for m in range(8):
                    for b in range(BL):
                        for blk in range(QT // BLK):
                            ps = pcps.tile([128, BLK], F32, tag="pc")
                            nc.tensor.matmul(
                                ps[:],
                                Wi0[:, m * 128:(m + 1) * 128],
                                XT[:, b * T + t0 + blk * BLK:
                                   b * T + t0 + (blk + 1) * BLK],
                                start=True, stop=True,
                            )
                            dst = gx[:, blk * BLK:(blk + 1) * BLK, 8 * m + b]
                            if (m * BL + b) % 2 == 0:
                                nc.vector.tensor_copy(dst, ps[:])
                            else:
                                nc.scalar.copy(dst, ps[:])

            def mm_group(g_ps, base, ks, rhss, first, last):
                # start=True only on the very first matmul touching this PSUM
                # bank (it clears has_written bank-wide); stop=True only on the
                # very last matmul of the bank's whole accumulation group.
                for m in range(8):
                    for j, k in enumerate(ks):
                        nc.tensor.matmul(
                            g_ps[:, 8 * m:8 * m + 8],
                            W1[:, (k * 8 + m) * 128:(k * 8 + m + 1) * 128]
                            if base == 1 else
                            W0[:, (k * 8 + m) * 128:(k * 8 + m + 1) * 128],
                            rhss[j],
                            start=(first and m == 0 and j == 0),
                            stop=(last and m == 7 and j == len(ks) - 1),
                        )

            def cell_actA(g_ps, sg):
                if "alltanh" in probe:
                    # timing probe: single-table, single-instruction actA
                    nc.scalar.activation(sg[:, 0:64], g_ps[:, 0:64], AF.Tanh)
                elif "tanhsplit" in probe:
                    # timing probe: same instr count as base, single table
                    nc.scalar.activation(sg[:, 0:48], g_ps[:, 0:48], AF.Tanh)
                    nc.scalar.activation(sg[:, 48:64], g_ps[:, 48:64], AF.Tanh)
                else:
                    nc.scalar.activation(sg[:, 0:48], g_ps[:, 0:48], AF.Sigmoid)
                    nc.scalar.activation(sg[:, 48:64], g_ps[:, 48:64], AF.Tanh)

            def cell_fmul(sg, cs):
                if "gpf" in probe:
                    nc.gpsimd.tensor_mul(cs[:], cs[:], sg[:, 16:32])
                else:
                    nc.vector.tensor_mul(cs[:], cs[:], sg[:, 16:32])

            def cell_ig(sg, cs):
                nc.vector.tensor_mul(sg[:, 64:80], sg[:, 0:16], sg[:, 48:64])
                nc.vector.tensor_add(cs[:], cs[:], sg[:, 64:80])

            def cell_actB(cs, tcs):
                nc.scalar.activation(tcs[:], cs[:], AF.Tanh)

            def cell_hmul(sg, tcs, hs):
                nc.vector.tensor_mul(hs[:], sg[:, 32:48], tcs[:])

            def cell(g_ps, sg, cs, tcs, hs):
                """gate view [128, 64] (i|f|o|g) -> update cs, hs."""
                if "noact" in probe:
                    nc.vector.tensor_copy(hs[:], g_ps[:, 0:16])
                    return
                cell_actA(g_ps, sg)
                cell_fmul(sg, cs)
                cell_ig(sg, cs)
                cell_actB(cs, tcs)
                cell_hmul(sg, tcs, hs)

            fcp = fcps_pool.tile([8, 2 * U], F32, tag="fc")

            def fc_mms(col, hbuf):
                # logits for one step into fcp[:, 2*col : 2*col+2]
                for cch in range(2):
                    nc.tensor.matmul(
                        fcp[0:8, 2 * col:2 * col + 2],
                        hbuf[:, 8 * cch:8 * cch + 8],
                        Wfc[:, 2 * cch:2 * cch + 2],
                        start=(cch == 0), stop=(cch == 1),
                    )

            def l1_mms_h(u):
                # gates1 for step u-1, recurrent part: bias seed + Whh1(h1(u-2)).
                # Independent of h0(u-1), so it keeps the PE busy while the
                # layer-0 act chain for step u-1 finishes.
                g1 = gps_pool.tile([128, 64], F32, tag="g1")
                nc.tensor.matmul(g1[:], ident[:], b1t[:], start=True, stop=False)
                hprev = h1bufs[u % 2]
                mm_group(g1, 1, [2, 3], [hprev[:, 0:8], hprev[:, 8:16]],
                         False, False)
                return g1

            def l1_mms_x(g1):
                # gates1 input part: Wih1(h0(u-1)) — needs the fresh h0.
                mm_group(g1, 1, [0, 1], [h0[:, 0:8], h0[:, 8:16]], False, True)

            def body(q, i, gx, first_global):
                """One U-step slab. Software pipeline: emits L0(s), L1(s-1),
                FC(s-2) per slot (s = global step). first_global skips the
                not-yet-existing lagged steps."""
                for u in range(U):
                    # ---- layer 0 matmuls for step u
                    g0 = gps_pool.tile([128, 64], F32, tag="g0")
                    gxs = gx[:, 0:1] if "staticgx" in probe else gx[:, ds(i * U + u, 1)]
                    nc.tensor.matmul(g0[:], ident[:], gxs,
                                     start=True, stop=False)
                    mm_group(g0, 0, [0, 1], [h0[:, 0:8], h0[:, 8:16]],
                             False, True)
                    # ---- layer 1 matmuls for step u-1
                    g1 = None
                    if not (first_global and u == 0):
                        g1 = l1_mms_h(u)
                        l1_mms_x(g1)
                    # ---- evacuate the previous slab's logits (before the
                    # FC write below overwrites col 0 of fcp)
                    if u == 2 and not first_global:
                        ob = osb_pool.tile([8, 2 * U], F32, tag="ob")
                        nc.vector.tensor_copy(ob[:], fcp[:])
                        nc.sync.dma_start(
                            out_d.ap()[:, ds(q * 2 * QT + i * 2 * U, 2 * U)],
                            ob[:])
                    # ---- FC for step u-2 (reads h1(u-2) = h1bufs[u % 2])
                    if not (first_global and u <= 1) and "nofc" not in probe:
                        fc_mms((u - 2) % U, h1bufs[u % 2])
                    # ---- cells, interleaved so each engine FIFO's wait
                    # gaps are filled with ready work from the other layer
                    sg0 = sg_pool.tile([128, 80], F32, tag="sg0")
                    if g1 is None or "noact" in probe:
                        cell(g0, sg0, c0, tc0, h0)
                        if g1 is not None:
                            sg1 = sg_pool.tile([128, 80], F32, tag="sg1")
                            cell(g1, sg1, c1, tc1, h1bufs[(u + 1) % 2])
                    else:
                        sg1 = sg_pool.tile([128, 80], F32, tag="sg1")
                        h1o = h1bufs[(u + 1) % 2]
                        cell_actA(g0, sg0)
                        cell_fmul(sg0, c0)
                        cell_actA(g1, sg1)
                        cell_ig(sg0, c0)
                        cell_fmul(sg1, c1)
                        cell_actB(c0, tc0)
                        cell_ig(sg1, c1)
                        cell_hmul(sg0, tc0, h0)
                        cell_actB(c1, tc1)
                        cell_hmul(sg1, tc1, h1o)

            def serial_quarter(q, gx):
                n_iter = QT // U
                if q == 0:
                    body(0, 0, gx, True)
                hints = ((mybir.EngineType.PE, mybir.EngineType.Activation,
                          mybir.EngineType.DVE) if "hint" in probe else ())
                with tc.For_i(1 if q == 0 else 0, n_iter,
                              hint_engines=hints) as i:
                    body(q, i, gx, False)

            def epilogue():
                # L1(T-1), FC(T-2), FC(T-1), final evacuation.
                g1 = l1_mms_h(0)        # u == 0 parity: reads h1(T-2) = bufs[0]
                l1_mms_x(g1)
                fc_mms(U - 2, h1bufs[0])
                sg1 = sg_pool.tile([128, 80], F32, tag="sg1")
                cell_actA(g1, sg1)
                cell_fmul(sg1, c1)
                cell_ig(sg1, c1)
                cell_actB(c1, tc1)
                cell_hmul(sg1, tc1, h1bufs[1])
                fc_mms(U - 1, h1bufs[1])
                ob = osb_pool.tile([8, 2 * U], F32, tag="ob")
                nc.vector.tensor_copy(ob[:], fcp[:])
                nc.sync.dma_start(out_d.ap()[:, ds(2 * T, 2 * U)], ob[:])

            body_reps = max(1, reps)
            if body_reps == 1:
                init_state()
                for q in range(NQ):
                    precompute_quarter(q, GX[q % 2])
                    serial_quarter(q, GX[q % 2])
                epilogue()
            else:
                with tc.For_i(0, body_reps) as _r:
                    init_state()
                    for q in range(NQ):
                        precompute_quarter(q, GX[q % 2])
                        serial_quarter(q, GX[q % 2])
                    epilogue()

    nc.compile()
    return nc


def _prep_inputs(inputs):
    perm = _gate_perm()
    f32 = np.float32
    Wh0 = np.asarray(inputs["W_hh0"], f32)[perm]          # [1024, 256]
    Wi0 = np.asarray(inputs["W_ih0"], f32)[perm]          # [1024, 4]
    b0 = (np.asarray(inputs["b_ih0"], f32) + np.asarray(inputs["b_hh0"], f32))[perm]
    Wi1 = np.asarray(inputs["W_ih1"], f32)[perm]          # [1024, 256]
    Wh1 = np.asarray(inputs["W_hh1"], f32)[perm]
    b1 = (np.asarray(inputs["b_ih1"], f32) + np.asarray(inputs["b_hh1"], f32))[perm]
    Wfc = np.asarray(inputs["W_fc"], f32)                 # [2, 256]
    bfc = np.asarray(inputs["b_fc"], f32)                 # [2]
    y = np.asarray(inputs["y"], f32)                      # [64, 2048]

    # W0 image: [128(p), 2(k), 8(m), 128(q)] with value Wh0[128m+q, 128k+p]
    w0img = np.ascontiguousarray(
        Wh0.reshape(8, 128, 2, 128).transpose(3, 2, 0, 1).reshape(128, 16 * 128)
    ).astype(nbf16)
    # W1 image: k in 0..3 -> k<2: Wih1 (contract h0), k>=2: Whh1 (contract h1)
    wi1_t = Wi1.reshape(8, 128, 2, 128).transpose(3, 2, 0, 1)    # [p, k, m, q]
    wh1_t = Wh1.reshape(8, 128, 2, 128).transpose(3, 2, 0, 1)
    w1img = np.ascontiguousarray(
        np.concatenate([wi1_t, wh1_t], axis=1).reshape(128, 32 * 128)
    ).astype(nbf16)
    # Wi0 image [5, 8*128]: rows 0:4 taps, row 4 bias
    wi0img = np.zeros((5, 8 * 128), np.float32)
    wi0img[0:4] = Wi0.reshape(8, 128, 4).transpose(2, 0, 1).reshape(4, 8 * 128)
    wi0img[4] = b0
    wi0img = wi0img.astype(nbf16)
    # Wfc image [128, 4]: [p, 2c+n] = Wfc[n, 128c+p]
    wfcimg = np.ascontiguousarray(
        Wfc.reshape(2, 2, 128).transpose(2, 1, 0).reshape(128, 4)
    ).astype(nbf16)
    # b1 tile [128, 64]: [p, 8c+b] = b1[128c+p]
    b1img = np.ascontiguousarray(
        np.repeat(b1.reshape(8, 128).T[:, :, None], 8, axis=2).reshape(128, 64)
    ).astype(nbf16)
    identimg = np.eye(128, dtype=np.float32).astype(nbf16)

    in_maps = []
    for c in range(N_CORES):
        in_maps.append({
            "w0img": w0img, "w1img": w1img, "wi0img": wi0img,
            "wfcimg": wfcimg, "b1img": b1img, "identimg": identimg,
            "yloc": np.ascontiguousarray(y[c * BL:(c + 1) * BL]),
        })
    return in_maps, bfc


def kernel(**inputs):
    if "nc" not in _compiled:
        _compiled["nc"] = build(1)
    nc = _compiled["nc"]
    in_maps, bfc = _prep_inputs(inputs)
    res = run_bass_kernel_spmd(nc, in_maps, core_ids=list(range(N_CORES)))
    out = np.concatenate(
        [r["out"][:, 2 * U:].reshape(BL, T, 2) for r in res.results], axis=0
    ).astype(np.float32)
    return out + bfc[None, None, :]

